# revision 21
# baseline (speedup 1.0000x reference)
"""Int8-dequant linear kernel for Trainium2 (8 NeuronCores, tensor-parallel).

Computes  y = x @ (qweight * weight_scale)^T + bias
  x:       [4096, 4096]  f32
  qweight: [16384, 4096] int8 (or int32)
  bias:    [16384]       f32
  y:       [4096, 16384] f32

Sharding: column-parallel over out_features — each of the 8 cores owns a
[2048, 4096] slice of qweight and the matching bias slice; x is replicated.
Each core computes its [4096, 2048] output slice; the host concatenates.

Math/layout choices:
  * weight_scale is folded into x on the host: y = (x*s) @ qw^T + bias.
  * Both matmul operands are cast to bf16. int8 weight values are exactly
    representable in bf16; x*s loses ~2^-9 relative — accumulation is fp32
    in PSUM, so the end-to-end relative error is ~1e-3.
  * Operands are pre-arranged host-side into partition-major layouts so the
    contraction dim (d = ko*128 + ki) lands on SBUF partitions (ki) and every
    DMA is >=4KB contiguous per partition.

Device kernel (per core): the whole weight shard lives in SBUF as bf16
([128, 32, 2048] = 128KB/partition). For each of 32 token tiles, stream the
x^T tile [128, 32, 128], run 32 (k) x 4 (n) matmuls of N=512 accumulating
into 4 PSUM banks, then evict with a fused psum+bias tensor_add and DMA out.
"""

import numpy as np
import ml_dtypes

import concourse.bass as bass
import concourse.mybir as mybir
import concourse.tile as tile
from concourse import bacc
from concourse.bass_utils import run_bass_kernel_spmd

N_CORES = 8
TOKENS, D_IN, D_OUT = 4096, 4096, 16384
O_SH = D_OUT // N_CORES  # 2048 out-features per core
P = 128
KO = D_IN // P  # 32 contraction chunks
MO = TOKENS // P  # 32 token tiles
N_FREE = 512  # matmul moving free dim == one PSUM bank of f32
N_TILES = O_SH // N_FREE  # 4

_cache: dict = {}


def _build_bass():
    bf16 = mybir.dt.bfloat16
    f32 = mybir.dt.float32
    # Bacc (not raw Bass): its compile() legalizes multi-wait instructions
    # via EventSemaphore hoisting — engine instructions only carry one
    # embedded sync-wait on trn2.
    nc = bacc.Bacc(
        "TRN2", target_bir_lowering=False, debug=False, num_devices=N_CORES
    )

    # xp[ki, mo, ko, t] = (x*scale)[mo*128+t, ko*128+ki] as bf16
    x_d = nc.dram_tensor("xp", (P, MO, KO, P), bf16, kind="ExternalInput")
    # wp[ki, ko, o] = qweight_shard[o, ko*128+ki] as bf16
    w_d = nc.dram_tensor("wp", (P, KO, O_SH), bf16, kind="ExternalInput")
    # bp[p, o] = bias_shard[o] replicated across partitions
    b_d = nc.dram_tensor("bp", (P, O_SH), f32, kind="ExternalInput")
    y_d = nc.dram_tensor("y", (TOKENS, O_SH), f32, kind="ExternalOutput")

    with tile.TileContext(nc) as tc:
        with (
            tc.tile_pool(name="wpool", bufs=KO - 4) as wpool,
            tc.tile_pool(name="wqpool", bufs=16) as wqpool,
            tc.tile_pool(name="xpool", bufs=4) as xpool,
            tc.tile_pool(name="bpool", bufs=1) as bpool,
            tc.tile_pool(name="opool", bufs=8) as opool,
            tc.tile_pool(name="psum", bufs=8, space="PSUM") as psum_pool,
        ):

            def load_x(m):
                # Two half-tiles per token tile so the first matmul group
                # of a tile only waits on the lo half's 0.5MB.
                lo = xpool.tile([P, KO // 2, P], bf16, tag="xlo", name=f"x_m{m}lo")
                hi = xpool.tile([P, KO // 2, P], bf16, tag="xhi", name=f"x_m{m}hi")
                nc.sync.dma_start(lo[:], x_d[:, m, : KO // 2])
                nc.sync.dma_start(hi[:], x_d[:, m, KO // 2 :])
                return (lo, hi)

            def alloc_psums(label):
                return [
                    psum_pool.tile([P, N_FREE], f32, tag="ps", name=f"ps_{label}_{n}")
                    for n in range(N_TILES)
                ]

            def rhs_of(ko, n):
                wt = w_tiles[ko]
                if isinstance(wt, list):  # quarter-split early chunks
                    return wt[n][:]
                return wt[:, n * N_FREE : (n + 1) * N_FREE]

            def mm_group(psums, x_parts, ko, start, stop):
                lhsT = x_parts[ko // (KO // 2)][:, ko % (KO // 2)]
                for n in range(N_TILES):
                    nc.tensor.matmul(
                        psums[n][:],
                        lhsT,
                        rhs_of(ko, n),
                        start=start,
                        stop=stop,
                    )

            def evict(psums, m):
                # Fused eviction out = psum + bias (bias replicated across
                # partitions); per-n stores so DMA overlaps later evictions.
                # Bacc's event-semaphore legalization handles the multi-wait
                # 3-operand tensor_tensor.
                for n in range(N_TILES):
                    sl = slice(n * N_FREE, (n + 1) * N_FREE)
                    o_sb = opool.tile([P, N_FREE], f32, tag="o", name=f"o_{m}_{n}")
                    nc.vector.tensor_add(o_sb[:], psums[n][:], bias_sb[:, sl])
                    nc.sync.dma_start(y_d[m * P : (m + 1) * P, sl], o_sb[:])

            # PE prewarm: the NEFF preamble + first DMAs take ~15us to
            # deliver the first operands; dummy matmuls on memset tiles
            # fill that idle window so the HAM clock-gate (K=4/8 cold ->
            # 8/8 after ~3.4us of activity) is warm when real data lands.
            g_lhs = bpool.tile([P, P], bf16)
            g_rhs = bpool.tile([P, N_FREE], bf16)
            nc.any.memset(g_lhs[:], 1.0)
            nc.any.memset(g_rhs[:], 1.0)
            ps_warm = psum_pool.tile([P, N_FREE], f32, tag="ps", name="ps_warm")
            for _ in range(12):
                nc.tensor.matmul(ps_warm[:], g_lhs[:], g_rhs[:], start=True, stop=True)

            # DMA startup on one HWDGE ring (two rings just split the 16
            # SDMA engines 50/50 per packet, slowing the critical path),
            # ordered by first use. w0-w3 are quarter-split so the first
            # matmuls wait on 128KB slices, not full 512KB chunks (early
            # DMA delivers only ~0.5MB in its first 5us). HBM per core is
            # ~358GB/s so the full weight load takes ~45us; the first two
            # token tiles are fused into one k-loop below so PE consumes
            # weight chunks slower (~1.7us/chunk) than DMA delivers them
            # and never stalls on the stream.
            def load_w(ko):
                wt = wpool.tile([P, O_SH], bf16, tag="w", name=f"w_{ko}")
                nc.sync.dma_start(wt[:], w_d[:, ko])
                return wt

            def load_w_quarters(ko):
                ts = []
                for n in range(N_TILES):
                    t = wqpool.tile([P, N_FREE], bf16, tag="wq", name=f"w{ko}q{n}")
                    nc.sync.dma_start(
                        t[:], w_d[:, ko, n * N_FREE : (n + 1) * N_FREE]
                    )
                    ts.append(t)
                return ts

            x_tiles = {}
            w_tiles = {}
            x0lo = xpool.tile([P, KO // 2, P], bf16, tag="xlo", name="x_m0lo")
            nc.sync.dma_start(x0lo[:], x_d[:, 0, : KO // 2])
            w_tiles[0] = load_w_quarters(0)
            x1lo = xpool.tile([P, KO // 2, P], bf16, tag="xlo", name="x_m1lo")
            nc.sync.dma_start(x1lo[:], x_d[:, 1, : KO // 2])
            w_tiles[1] = load_w_quarters(1)
            w_tiles[2] = load_w_quarters(2)
            w_tiles[3] = load_w_quarters(3)
            x0hi = xpool.tile([P, KO // 2, P], bf16, tag="xhi", name="x_m0hi")
            nc.sync.dma_start(x0hi[:], x_d[:, 0, KO // 2 :])
            x1hi = xpool.tile([P, KO // 2, P], bf16, tag="xhi", name="x_m1hi")
            nc.sync.dma_start(x1hi[:], x_d[:, 1, KO // 2 :])
            x_tiles[0] = (x0lo, x0hi)
            x_tiles[1] = (x1lo, x1hi)
            for ko in range(4, 9):
                w_tiles[ko] = load_w(ko)
            bias_sb = bpool.tile([P, O_SH], f32)
            nc.sync.dma_start(bias_sb[:], b_d[:])
            for ko in range(9, KO):
                w_tiles[ko] = load_w(ko)

            # Fused pair phase: m=0 and m=1 share one k-loop (8 PSUM banks).
            ps0, ps1 = alloc_psums("a"), alloc_psums("b")
            for ko in range(KO):
                mm_group(ps0, x_tiles[0], ko, start=(ko == 0), stop=(ko == KO - 1))
                mm_group(ps1, x_tiles[1], ko, start=(ko == 0), stop=(ko == KO - 1))
            evict(ps0, 0)
            evict(ps1, 1)

            # Steady state: one m per k-loop, 4+4 PSUM double buffering.
            for m in range(2, MO):
                x_sb = load_x(m)
                psums = alloc_psums("s")
                if m < MO - 1:
                    for ko in range(KO):
                        mm_group(
                            psums, x_sb, ko, start=(ko == 0), stop=(ko == KO - 1)
                        )
                    evict(psums, m)
                else:
                    # Last tile: n-outer/ko-inner so each bank's eviction +
                    # store overlaps the remaining banks' matmuls — trims
                    # the end-of-kernel serial evict/store chain.
                    for n in range(N_TILES):
                        for ko in range(KO):
                            lhsT = x_sb[ko // (KO // 2)][:, ko % (KO // 2)]
                            nc.tensor.matmul(
                                psums[n][:],
                                lhsT,
                                rhs_of(ko, n),
                                start=(ko == 0),
                                stop=(ko == KO - 1),
                            )
                        sl = slice(n * N_FREE, (n + 1) * N_FREE)
                        o_sb = opool.tile(
                            [P, N_FREE], f32, tag="o", name=f"o_last_{n}"
                        )
                        nc.vector.tensor_add(o_sb[:], psums[n][:], bias_sb[:, sl])
                        nc.sync.dma_start(y_d[m * P : (m + 1) * P, sl], o_sb[:])

    nc.compile()
    return nc


def _prep_in_maps(x, qweight, weight_scale, bias):
    bf16 = ml_dtypes.bfloat16
    scale = np.float32(np.asarray(weight_scale))
    xs = (np.asarray(x, dtype=np.float32) * scale).astype(bf16)  # [T, D]
    # [T=(mo t), D=(ko ki)] -> [ki, mo, ko, t]
    x_prep = np.ascontiguousarray(xs.reshape(MO, P, KO, P).transpose(3, 0, 2, 1))

    qw = np.asarray(qweight)
    b = np.asarray(bias, dtype=np.float32)
    in_maps = []
    for c in range(N_CORES):
        qc = qw[c * O_SH : (c + 1) * O_SH, :].astype(bf16)  # [O_SH, D], exact
        # [D=(ko ki), O] -> [ki, ko, o]
        w_prep = np.ascontiguousarray(qc.T.reshape(KO, P, O_SH).transpose(1, 0, 2))
        b_prep = np.ascontiguousarray(
            np.broadcast_to(b[c * O_SH : (c + 1) * O_SH], (P, O_SH))
        )
        in_maps.append({"xp": x_prep, "wp": w_prep, "bp": b_prep})
    return in_maps


def _run(inputs, **kwargs):
    if "nc" not in _cache:
        _cache["nc"] = _build_bass()
    nc = _cache["nc"]
    in_maps = _prep_in_maps(**inputs)
    res = run_bass_kernel_spmd(nc, in_maps, core_ids=list(range(N_CORES)), **kwargs)
    y = np.concatenate([res.results[c]["y"] for c in range(N_CORES)], axis=1)
    return y, res


def kernel(**inputs) -> np.ndarray:
    y, _ = _run(inputs)
    return y


# revision 25
# speedup vs baseline: 1.0113x; 1.0113x over previous
"""Int8-dequant linear kernel for Trainium2 (8 NeuronCores, tensor-parallel).

Computes  y = x @ (qweight * weight_scale)^T + bias
  x:       [4096, 4096]  f32
  qweight: [16384, 4096] int8 (or int32)
  bias:    [16384]       f32
  y:       [4096, 16384] f32

Sharding: column-parallel over out_features — each of the 8 cores owns a
[2048, 4096] slice of qweight and the matching bias slice; x is replicated.
Each core computes its [4096, 2048] output slice; the host concatenates.

Math/layout choices:
  * weight_scale is folded into x on the host: y = (x*s) @ qw^T + bias.
  * Both matmul operands are cast to bf16. int8 weight values are exactly
    representable in bf16; x*s loses ~2^-9 relative — accumulation is fp32
    in PSUM, so the end-to-end relative error is ~1e-3.
  * Operands are pre-arranged host-side into partition-major layouts so the
    contraction dim (d = ko*128 + ki) lands on SBUF partitions (ki) and every
    DMA is >=4KB contiguous per partition.

Device kernel (per core): the whole weight shard lives in SBUF as bf16
([128, 32, 2048] = 128KB/partition). For each of 32 token tiles, stream the
x^T tile [128, 32, 128], run 32 (k) x 4 (n) matmuls of N=512 accumulating
into 4 PSUM banks, then evict with a fused psum+bias tensor_add and DMA out.
"""

import numpy as np
import ml_dtypes

import concourse.bass as bass
import concourse.mybir as mybir
import concourse.tile as tile
from concourse import bacc
from concourse.bass_utils import run_bass_kernel_spmd

N_CORES = 8
TOKENS, D_IN, D_OUT = 4096, 4096, 16384
O_SH = D_OUT // N_CORES  # 2048 out-features per core
P = 128
KO = D_IN // P  # 32 contraction chunks
MO = TOKENS // P  # 32 token tiles
N_FREE = 512  # matmul moving free dim == one PSUM bank of f32
N_TILES = O_SH // N_FREE  # 4

_cache: dict = {}


def _build_bass():
    bf16 = mybir.dt.bfloat16
    f32 = mybir.dt.float32
    # Bacc (not raw Bass): its compile() legalizes multi-wait instructions
    # via EventSemaphore hoisting — engine instructions only carry one
    # embedded sync-wait on trn2.
    nc = bacc.Bacc(
        "TRN2", target_bir_lowering=False, debug=False, num_devices=N_CORES
    )

    i8 = mybir.dt.int8
    # xp[ki, mo, ko, t] = (x*scale)[mo*128+t, ko*128+ki] as bf16
    x_d = nc.dram_tensor("xp", (P, MO, KO, P), bf16, kind="ExternalInput")
    # wp8[ki, ko, o] = qweight_shard[o, ko*128+ki] as int8 — shipped raw
    # (half the HBM bytes of bf16) and dequantized to bf16 on-device.
    w_d = nc.dram_tensor("wp8", (P, KO, O_SH), i8, kind="ExternalInput")
    # bp[p, o] = bias_shard[o] replicated across partitions
    b_d = nc.dram_tensor("bp", (P, O_SH), f32, kind="ExternalInput")
    y_d = nc.dram_tensor("y", (TOKENS, O_SH), f32, kind="ExternalOutput")

    with tile.TileContext(nc) as tc:
        with (
            tc.tile_pool(name="wpool", bufs=KO - 4) as wpool,
            tc.tile_pool(name="wqpool", bufs=16) as wqpool,
            tc.tile_pool(name="spool", bufs=4) as spool,
            tc.tile_pool(name="xpool", bufs=4) as xpool,
            tc.tile_pool(name="bpool", bufs=1) as bpool,
            tc.tile_pool(name="opool", bufs=6) as opool,
            tc.tile_pool(name="psum", bufs=8, space="PSUM") as psum_pool,
        ):

            def load_x(m):
                # Two half-tiles per token tile so the first matmul group
                # of a tile only waits on the lo half's 0.5MB.
                lo = xpool.tile([P, KO // 2, P], bf16, tag="xlo", name=f"x_m{m}lo")
                hi = xpool.tile([P, KO // 2, P], bf16, tag="xhi", name=f"x_m{m}hi")
                nc.sync.dma_start(lo[:], x_d[:, m, : KO // 2])
                nc.sync.dma_start(hi[:], x_d[:, m, KO // 2 :])
                return (lo, hi)

            def alloc_psums(label):
                return [
                    psum_pool.tile([P, N_FREE], f32, tag="ps", name=f"ps_{label}_{n}")
                    for n in range(N_TILES)
                ]

            def rhs_of(ko, n):
                wt = w_tiles[ko]
                if isinstance(wt, list):  # quarter-split early chunks
                    return wt[n][:]
                return wt[:, n * N_FREE : (n + 1) * N_FREE]

            def mm_group(psums, x_parts, ko, start, stop):
                lhsT = x_parts[ko // (KO // 2)][:, ko % (KO // 2)]
                for n in range(N_TILES):
                    nc.tensor.matmul(
                        psums[n][:],
                        lhsT,
                        rhs_of(ko, n),
                        start=start,
                        stop=stop,
                    )

            def evict(psums, m):
                # Fused eviction out = psum + bias (bias replicated across
                # partitions); per-n stores so DMA overlaps later evictions.
                # Bacc's event-semaphore legalization handles the multi-wait
                # 3-operand tensor_tensor.
                for n in range(N_TILES):
                    sl = slice(n * N_FREE, (n + 1) * N_FREE)
                    o_sb = opool.tile([P, N_FREE], f32, tag="o", name=f"o_{m}_{n}")
                    nc.vector.tensor_add(o_sb[:], psums[n][:], bias_sb[:, sl])
                    nc.sync.dma_start(y_d[m * P : (m + 1) * P, sl], o_sb[:])

            # Weight path: DMA raw int8 chunks (0.25MB each — half the HBM
            # bytes of bf16, critical in the slow early-DMA window), then
            # dequantize to resident bf16 tiles. Dequant alternates between
            # VectorE and ScalarE (~2us per [128,2048] chunk each, so the
            # alternating pair sustains ~1.1us/chunk — faster than the
            # fused pair phase consumes chunks at ~1.73us/chunk). int8
            # values are exactly representable in bf16; weight_scale is
            # folded into x on the host.
            def stage_w(ko):
                s = spool.tile([P, O_SH], i8, tag="wi8", name=f"wi8_{ko}")
                nc.sync.dma_start(s[:], w_d[:, ko])
                return s

            def dequant_full(ko, s):
                wt = wpool.tile([P, O_SH], bf16, tag="w", name=f"w_{ko}")
                if ko % 2 == 0:
                    nc.vector.tensor_copy(wt[:], s[:])
                else:
                    nc.scalar.copy(wt[:], s[:])
                return wt

            def dequant_quarters(ko, s):
                # Early chunks as 4 quarter tiles so the first matmuls wait
                # on a [128,512] dequant, not the full chunk.
                ts = []
                for n in range(N_TILES):
                    t = wqpool.tile([P, N_FREE], bf16, tag="wq", name=f"w{ko}q{n}")
                    sl = slice(n * N_FREE, (n + 1) * N_FREE)
                    if n % 2 == 0:
                        nc.vector.tensor_copy(t[:], s[:, sl])
                    else:
                        nc.scalar.copy(t[:], s[:, sl])
                    ts.append(t)
                return ts

            x_tiles = {}
            w_tiles = {}
            x0lo = xpool.tile([P, KO // 2, P], bf16, tag="xlo", name="x_m0lo")
            nc.sync.dma_start(x0lo[:], x_d[:, 0, : KO // 2])
            s0 = stage_w(0)
            x1lo = xpool.tile([P, KO // 2, P], bf16, tag="xlo", name="x_m1lo")
            nc.sync.dma_start(x1lo[:], x_d[:, 1, : KO // 2])
            s1 = stage_w(1)
            w_tiles[0] = dequant_quarters(0, s0)
            w_tiles[1] = dequant_quarters(1, s1)
            s2 = stage_w(2)
            s3 = stage_w(3)
            w_tiles[2] = dequant_quarters(2, s2)
            w_tiles[3] = dequant_quarters(3, s3)
            x0hi = xpool.tile([P, KO // 2, P], bf16, tag="xhi", name="x_m0hi")
            nc.sync.dma_start(x0hi[:], x_d[:, 0, KO // 2 :])
            x1hi = xpool.tile([P, KO // 2, P], bf16, tag="xhi", name="x_m1hi")
            nc.sync.dma_start(x1hi[:], x_d[:, 1, KO // 2 :])
            x_tiles[0] = (x0lo, x0hi)
            x_tiles[1] = (x1lo, x1hi)
            for ko in range(4, 9):
                w_tiles[ko] = dequant_full(ko, stage_w(ko))
            bias_sb = bpool.tile([P, O_SH], f32)
            nc.sync.dma_start(bias_sb[:], b_d[:])
            for ko in range(9, KO):
                w_tiles[ko] = dequant_full(ko, stage_w(ko))

            # Fused pair phase: m=0 and m=1 share one k-loop (8 PSUM banks).
            ps0, ps1 = alloc_psums("a"), alloc_psums("b")
            for ko in range(KO):
                mm_group(ps0, x_tiles[0], ko, start=(ko == 0), stop=(ko == KO - 1))
                mm_group(ps1, x_tiles[1], ko, start=(ko == 0), stop=(ko == KO - 1))
            evict(ps0, 0)
            evict(ps1, 1)

            # Steady state: one m per k-loop, 4+4 PSUM double buffering.
            for m in range(2, MO):
                x_sb = load_x(m)
                psums = alloc_psums("s")
                if m < MO - 1:
                    for ko in range(KO):
                        mm_group(
                            psums, x_sb, ko, start=(ko == 0), stop=(ko == KO - 1)
                        )
                    evict(psums, m)
                else:
                    # Last tile: n-outer/ko-inner so each bank's eviction +
                    # store overlaps the remaining banks' matmuls — trims
                    # the end-of-kernel serial evict/store chain.
                    for n in range(N_TILES):
                        for ko in range(KO):
                            lhsT = x_sb[ko // (KO // 2)][:, ko % (KO // 2)]
                            nc.tensor.matmul(
                                psums[n][:],
                                lhsT,
                                rhs_of(ko, n),
                                start=(ko == 0),
                                stop=(ko == KO - 1),
                            )
                        sl = slice(n * N_FREE, (n + 1) * N_FREE)
                        o_sb = opool.tile(
                            [P, N_FREE], f32, tag="o", name=f"o_last_{n}"
                        )
                        nc.vector.tensor_add(o_sb[:], psums[n][:], bias_sb[:, sl])
                        nc.sync.dma_start(y_d[m * P : (m + 1) * P, sl], o_sb[:])

    nc.compile()
    return nc


def _prep_in_maps(x, qweight, weight_scale, bias):
    bf16 = ml_dtypes.bfloat16
    scale = np.float32(np.asarray(weight_scale))
    xs = (np.asarray(x, dtype=np.float32) * scale).astype(bf16)  # [T, D]
    # [T=(mo t), D=(ko ki)] -> [ki, mo, ko, t]
    x_prep = np.ascontiguousarray(xs.reshape(MO, P, KO, P).transpose(3, 0, 2, 1))

    qw = np.asarray(qweight).astype(np.int8)  # values in [-128,127]
    b = np.asarray(bias, dtype=np.float32)
    in_maps = []
    for c in range(N_CORES):
        qc = qw[c * O_SH : (c + 1) * O_SH, :]  # [O_SH, D]
        # [D=(ko ki), O] -> [ki, ko, o]
        w_prep = np.ascontiguousarray(qc.T.reshape(KO, P, O_SH).transpose(1, 0, 2))
        b_prep = np.ascontiguousarray(
            np.broadcast_to(b[c * O_SH : (c + 1) * O_SH], (P, O_SH))
        )
        in_maps.append({"xp": x_prep, "wp8": w_prep, "bp": b_prep})
    return in_maps


def _run(inputs, **kwargs):
    if "nc" not in _cache:
        _cache["nc"] = _build_bass()
    nc = _cache["nc"]
    in_maps = _prep_in_maps(**inputs)
    res = run_bass_kernel_spmd(nc, in_maps, core_ids=list(range(N_CORES)), **kwargs)
    y = np.concatenate([res.results[c]["y"] for c in range(N_CORES)], axis=1)
    return y, res


def kernel(**inputs) -> np.ndarray:
    y, _ = _run(inputs)
    return y


# revision 28
# speedup vs baseline: 1.0131x; 1.0018x over previous
"""Int8-dequant linear kernel for Trainium2 (8 NeuronCores, tensor-parallel).

Computes  y = x @ (qweight * weight_scale)^T + bias
  x:       [4096, 4096]  f32
  qweight: [16384, 4096] int8 (or int32)
  bias:    [16384]       f32
  y:       [4096, 16384] f32

Sharding: column-parallel over out_features — each of the 8 cores owns a
[2048, 4096] slice of qweight and the matching bias slice; x is replicated.
Each core computes its [4096, 2048] output slice; the host concatenates.

Math/layout choices:
  * weight_scale is folded into x on the host: y = (x*s) @ qw^T + bias.
  * The matmul runs in bf16. int8 weight values are exactly representable
    in bf16; x*s loses ~2^-9 relative — accumulation is fp32 in PSUM, so
    the end-to-end relative error is ~1e-3.
  * Weights ship to the device as raw int8 (half the HBM bytes) and are
    dequantized to resident bf16 SBUF tiles by VectorE/ScalarE copies.
  * Operands are pre-arranged host-side into partition-major layouts so the
    contraction dim (d = ko*128 + ki) lands on SBUF partitions (ki) and every
    DMA is >=2KB contiguous per partition.

Device kernel (per core): the whole weight shard lives in SBUF as bf16
([128, 32, 2048] = 128KB/partition). For each of 32 token tiles, stream the
x^T tile [128, 32, 128], run 32 (k) x 4 (n) matmuls of N=512 accumulating
into 4 PSUM banks, then evict with a fused psum+bias tensor_add and DMA out.
The first two token tiles share one k-loop (8 PSUM banks) so the PE outlasts
the ~22us weight stream without stalling; the last tile runs n-outer so its
evictions overlap its matmuls. Measured: ~905us/core HW time at ~97% PE
occupancy (bf16 roofline for the 4096 N=512 matmuls is ~884us), rel err
~1.7e-3.
"""

import numpy as np
import ml_dtypes

import concourse.mybir as mybir
import concourse.tile as tile
from concourse import bacc
from concourse.bass_utils import run_bass_kernel_spmd

N_CORES = 8
TOKENS, D_IN, D_OUT = 4096, 4096, 16384
O_SH = D_OUT // N_CORES  # 2048 out-features per core
P = 128
KO = D_IN // P  # 32 contraction chunks
MO = TOKENS // P  # 32 token tiles
N_FREE = 512  # matmul moving free dim == one PSUM bank of f32
N_TILES = O_SH // N_FREE  # 4

_cache: dict = {}


def _build_bass():
    bf16 = mybir.dt.bfloat16
    f32 = mybir.dt.float32
    # Bacc (not raw Bass): its compile() legalizes multi-wait instructions
    # via EventSemaphore hoisting — engine instructions only carry one
    # embedded sync-wait on trn2.
    nc = bacc.Bacc(
        "TRN2", target_bir_lowering=False, debug=False, num_devices=N_CORES
    )

    i8 = mybir.dt.int8
    # xp[ki, mo, ko, t] = (x*scale)[mo*128+t, ko*128+ki] as bf16
    x_d = nc.dram_tensor("xp", (P, MO, KO, P), bf16, kind="ExternalInput")
    # wp8[ki, ko, o] = qweight_shard[o, ko*128+ki] as int8 — shipped raw
    # (half the HBM bytes of bf16) and dequantized to bf16 on-device.
    w_d = nc.dram_tensor("wp8", (P, KO, O_SH), i8, kind="ExternalInput")
    # bp[p, o] = bias_shard[o] replicated across partitions
    b_d = nc.dram_tensor("bp", (P, O_SH), f32, kind="ExternalInput")
    y_d = nc.dram_tensor("y", (TOKENS, O_SH), f32, kind="ExternalOutput")

    with tile.TileContext(nc) as tc:
        with (
            tc.tile_pool(name="wpool", bufs=KO - 4) as wpool,
            tc.tile_pool(name="wqpool", bufs=16) as wqpool,
            tc.tile_pool(name="spool", bufs=4) as spool,
            tc.tile_pool(name="xpool", bufs=4) as xpool,
            tc.tile_pool(name="bpool", bufs=1) as bpool,
            tc.tile_pool(name="opool", bufs=6) as opool,
            tc.tile_pool(name="psum", bufs=8, space="PSUM") as psum_pool,
        ):

            def load_x(m):
                # Two half-tiles per token tile so the first matmul group
                # of a tile only waits on the lo half's 0.5MB.
                lo = xpool.tile([P, KO // 2, P], bf16, tag="xlo", name=f"x_m{m}lo")
                hi = xpool.tile([P, KO // 2, P], bf16, tag="xhi", name=f"x_m{m}hi")
                nc.sync.dma_start(lo[:], x_d[:, m, : KO // 2])
                nc.sync.dma_start(hi[:], x_d[:, m, KO // 2 :])
                return (lo, hi)

            def alloc_psums(label):
                return [
                    psum_pool.tile([P, N_FREE], f32, tag="ps", name=f"ps_{label}_{n}")
                    for n in range(N_TILES)
                ]

            def rhs_of(ko, n):
                wt = w_tiles[ko]
                if isinstance(wt, list):  # quarter-split early chunks
                    return wt[n][:]
                return wt[:, n * N_FREE : (n + 1) * N_FREE]

            def mm_group(psums, x_parts, ko, start, stop):
                lhsT = x_parts[ko // (KO // 2)][:, ko % (KO // 2)]
                for n in range(N_TILES):
                    nc.tensor.matmul(
                        psums[n][:],
                        lhsT,
                        rhs_of(ko, n),
                        start=start,
                        stop=stop,
                    )

            def evict(psums, m):
                # Fused eviction out = psum + bias (bias replicated across
                # partitions); per-n stores so DMA overlaps later evictions.
                # Bacc's event-semaphore legalization handles the multi-wait
                # 3-operand tensor_tensor.
                for n in range(N_TILES):
                    sl = slice(n * N_FREE, (n + 1) * N_FREE)
                    o_sb = opool.tile([P, N_FREE], f32, tag="o", name=f"o_{m}_{n}")
                    nc.vector.tensor_add(o_sb[:], psums[n][:], bias_sb[:, sl])
                    nc.sync.dma_start(y_d[m * P : (m + 1) * P, sl], o_sb[:])

            # Weight path: DMA raw int8 chunks (0.25MB each — half the HBM
            # bytes of bf16, critical in the slow early-DMA window), then
            # dequantize to resident bf16 tiles. Dequant alternates between
            # VectorE and ScalarE (~2us per [128,2048] chunk each, so the
            # alternating pair sustains ~1.1us/chunk — faster than the
            # fused pair phase consumes chunks at ~1.73us/chunk). int8
            # values are exactly representable in bf16; weight_scale is
            # folded into x on the host.
            def stage_w(ko):
                s = spool.tile([P, O_SH], i8, tag="wi8", name=f"wi8_{ko}")
                nc.sync.dma_start(s[:], w_d[:, ko])
                return s

            def dequant_full(ko, s):
                wt = wpool.tile([P, O_SH], bf16, tag="w", name=f"w_{ko}")
                if ko % 2 == 0:
                    nc.vector.tensor_copy(wt[:], s[:])
                else:
                    nc.scalar.copy(wt[:], s[:])
                return wt

            def dequant_quarters(ko, s):
                # Early chunks as 4 quarter tiles so the first matmuls wait
                # on a [128,512] dequant, not the full chunk.
                ts = []
                for n in range(N_TILES):
                    t = wqpool.tile([P, N_FREE], bf16, tag="wq", name=f"w{ko}q{n}")
                    sl = slice(n * N_FREE, (n + 1) * N_FREE)
                    if n % 2 == 0:
                        nc.vector.tensor_copy(t[:], s[:, sl])
                    else:
                        nc.scalar.copy(t[:], s[:, sl])
                    ts.append(t)
                return ts

            x_tiles = {}
            w_tiles = {}
            # int8 stage of w0 goes FIRST: its dequant (the serial step the
            # first matmul waits on) then overlaps x0lo's larger DMA
            # instead of following it.
            s0 = stage_w(0)
            x0lo = xpool.tile([P, KO // 2, P], bf16, tag="xlo", name="x_m0lo")
            nc.sync.dma_start(x0lo[:], x_d[:, 0, : KO // 2])
            w_tiles[0] = dequant_quarters(0, s0)
            s1 = stage_w(1)
            x1lo = xpool.tile([P, KO // 2, P], bf16, tag="xlo", name="x_m1lo")
            nc.sync.dma_start(x1lo[:], x_d[:, 1, : KO // 2])
            w_tiles[1] = dequant_quarters(1, s1)
            s2 = stage_w(2)
            s3 = stage_w(3)
            w_tiles[2] = dequant_quarters(2, s2)
            w_tiles[3] = dequant_quarters(3, s3)
            x0hi = xpool.tile([P, KO // 2, P], bf16, tag="xhi", name="x_m0hi")
            nc.sync.dma_start(x0hi[:], x_d[:, 0, KO // 2 :])
            x1hi = xpool.tile([P, KO // 2, P], bf16, tag="xhi", name="x_m1hi")
            nc.sync.dma_start(x1hi[:], x_d[:, 1, KO // 2 :])
            x_tiles[0] = (x0lo, x0hi)
            x_tiles[1] = (x1lo, x1hi)
            for ko in range(4, 9):
                w_tiles[ko] = dequant_full(ko, stage_w(ko))
            bias_sb = bpool.tile([P, O_SH], f32)
            nc.sync.dma_start(bias_sb[:], b_d[:])
            for ko in range(9, KO):
                w_tiles[ko] = dequant_full(ko, stage_w(ko))

            # Fused pair phase: m=0 and m=1 share one k-loop (8 PSUM banks).
            ps0, ps1 = alloc_psums("a"), alloc_psums("b")
            for ko in range(KO):
                mm_group(ps0, x_tiles[0], ko, start=(ko == 0), stop=(ko == KO - 1))
                mm_group(ps1, x_tiles[1], ko, start=(ko == 0), stop=(ko == KO - 1))
            evict(ps0, 0)
            evict(ps1, 1)

            # Steady state: one m per k-loop, 4+4 PSUM double buffering.
            for m in range(2, MO):
                x_sb = load_x(m)
                psums = alloc_psums("s")
                if m < MO - 1:
                    for ko in range(KO):
                        mm_group(
                            psums, x_sb, ko, start=(ko == 0), stop=(ko == KO - 1)
                        )
                    evict(psums, m)
                else:
                    # Last tile: n-outer/ko-inner so each bank's eviction +
                    # store overlaps the remaining banks' matmuls — trims
                    # the end-of-kernel serial evict/store chain.
                    for n in range(N_TILES):
                        for ko in range(KO):
                            lhsT = x_sb[ko // (KO // 2)][:, ko % (KO // 2)]
                            nc.tensor.matmul(
                                psums[n][:],
                                lhsT,
                                rhs_of(ko, n),
                                start=(ko == 0),
                                stop=(ko == KO - 1),
                            )
                        sl = slice(n * N_FREE, (n + 1) * N_FREE)
                        o_sb = opool.tile(
                            [P, N_FREE], f32, tag="o", name=f"o_last_{n}"
                        )
                        nc.vector.tensor_add(o_sb[:], psums[n][:], bias_sb[:, sl])
                        nc.sync.dma_start(y_d[m * P : (m + 1) * P, sl], o_sb[:])

    nc.compile()
    return nc


def _prep_in_maps(x, qweight, weight_scale, bias):
    bf16 = ml_dtypes.bfloat16
    scale = np.float32(np.asarray(weight_scale))
    xs = (np.asarray(x, dtype=np.float32) * scale).astype(bf16)  # [T, D]
    # [T=(mo t), D=(ko ki)] -> [ki, mo, ko, t]
    x_prep = np.ascontiguousarray(xs.reshape(MO, P, KO, P).transpose(3, 0, 2, 1))

    qw = np.asarray(qweight).astype(np.int8)  # values in [-128,127]
    b = np.asarray(bias, dtype=np.float32)
    in_maps = []
    for c in range(N_CORES):
        qc = qw[c * O_SH : (c + 1) * O_SH, :]  # [O_SH, D]
        # [D=(ko ki), O] -> [ki, ko, o]
        w_prep = np.ascontiguousarray(qc.T.reshape(KO, P, O_SH).transpose(1, 0, 2))
        b_prep = np.ascontiguousarray(
            np.broadcast_to(b[c * O_SH : (c + 1) * O_SH], (P, O_SH))
        )
        in_maps.append({"xp": x_prep, "wp8": w_prep, "bp": b_prep})
    return in_maps


def _run(inputs, **kwargs):
    if "nc" not in _cache:
        _cache["nc"] = _build_bass()
    nc = _cache["nc"]
    in_maps = _prep_in_maps(**inputs)
    res = run_bass_kernel_spmd(nc, in_maps, core_ids=list(range(N_CORES)), **kwargs)
    y = np.concatenate([res.results[c]["y"] for c in range(N_CORES)], axis=1)
    return y, res


def kernel(**inputs) -> np.ndarray:
    y, _ = _run(inputs)
    return y
